# revision 13
# baseline (speedup 1.0000x reference)
"""MoE (8 experts, top-2, swiglu) Trainium2 kernel.

Strategy: expert-parallel across 8 NeuronCores — core e holds expert e's
weights (w1[e]: 32MB, w2[e]: 16MB) and computes that expert's contribution
for ALL 128 tokens densely; the per-token routing coefficient (0 for
unrouted tokens) is computed on-device from the routing logits and applied
to the expert output. The host sums the 8 partial outputs (the "combine").

Per-core device program (all fp32):
  MM1:  h[t, o-blk]   += hsT[k]^T @ w1T[k, o-blk]     (K=hidden, 8 chunks)
  swiglu: act = silu(h[:, :4096]) * h[:, 4096:]
  PE-transpose act -> actT [i, t]
  MM2:  y[t, h-blk]   += actT[ki]^T @ w2T[ki, h-blk]  (K=inter, 32 chunks)
  y *= coef (per-token routing coefficient, reduce from logits on device)

Weights are pre-arranged on the host into the exact SBUF image layouts so
every weight DMA is a fully contiguous >=2MB transfer.
"""

import numpy as np

import concourse.bass as bass
import concourse.bacc as bacc
import concourse.mybir as mybir
from concourse.tile import TileContext
from concourse.bass_utils import run_bass_kernel_spmd
from concourse.masks import make_identity

TOKENS = 128
HIDDEN = 1024
INTER = 4096
NEXP = 8
NCORES = 8

KH = HIDDEN // 128          # 8   hidden contraction chunks
IB = INTER // 512           # 8   i-blocks of 512 (each needs up + gate o-block)
KI = INTER // 128           # 32  inter contraction chunks
HB = 4                      # output h blocks
HBW = HIDDEN // HB          # 256

F32 = mybir.dt.float32
# dtype used for matmul operands; float32 = exact (4 cyc/row),
# float32r = fast single-pass mode (1 cyc/row at N>=256), lower precision.
MM_DT = mybir.dt.float32r


def _mm(ap):
    return ap


def build_bass(loop_n: int = 1):
    """loop_n > 1 wraps the body in a hardware loop (benchmarking only)."""
    import contextlib

    nc = bacc.Bacc(None, target_bir_lowering=False)

    hst = nc.declare_dram_parameter("hst", [128, KH, TOKENS], MM_DT, isOutput=False)
    w1s = nc.declare_dram_parameter("w1s", [IB, 128, 2, KH, 512], MM_DT, isOutput=False)
    w2s = nc.declare_dram_parameter("w2s", [HB, 128, KI, HBW], MM_DT, isOutput=False)
    routing = nc.declare_dram_parameter("routing", [128, NEXP], F32, isOutput=False)
    rlogit = nc.declare_dram_parameter("rlogit", [128, 1], F32, isOutput=False)
    outp = nc.declare_dram_parameter("outp", [128, HIDDEN], F32, isOutput=True)

    with TileContext(nc) as tc:
        with (
            tc.tile_pool(name="singles", bufs=1) as singles,
            tc.tile_pool(name="small", bufs=1) as small,
            tc.tile_pool(name="w1pool", bufs=2) as w1pool,
            tc.tile_pool(name="w2pool", bufs=2) as w2pool,
            tc.tile_pool(name="actpool", bufs=3) as actpool,
            tc.tile_pool(name="outpool", bufs=2) as outpool,
            tc.tile_pool(name="psum_u", bufs=2, space="PSUM") as psum_u,
            tc.tile_pool(name="psum_g", bufs=2, space="PSUM") as psum_g,
            tc.tile_pool(name="psum_t", bufs=2, space="PSUM") as psum_t,
            tc.tile_pool(name="psum_y", bufs=2, space="PSUM") as psum_y,
            tc.For_i(0, loop_n, 1) if loop_n > 1 else contextlib.nullcontext(),
        ):
            ident = singles.tile([128, 128], F32)
            make_identity(nc, ident)

            hst_sb = singles.tile([128, KH, TOKENS], MM_DT)
            nc.sync.dma_start(out=hst_sb, in_=hst[:])

            # ---- routing coefficient for this core's expert ----
            r_sb = small.tile([128, NEXP], F32)
            nc.sync.dma_start(out=r_sb, in_=routing[:])
            rl_sb = small.tile([128, 1], F32)
            nc.sync.dma_start(out=rl_sb, in_=rlogit[:])

            m1 = small.tile([128, 1], F32)
            nc.vector.reduce_max(out=m1, in_=r_sb, axis=mybir.AxisListType.X)
            # mask out (one) max element, take max again -> second max
            mask = small.tile([128, NEXP], F32)
            nc.vector.tensor_scalar(
                out=mask, in0=r_sb, scalar1=m1, scalar2=None,
                op0=mybir.AluOpType.is_ge,
            )
            negmask = small.tile([128, NEXP], F32)
            nc.vector.tensor_scalar(
                out=negmask, in0=mask, scalar1=-1.0e30, scalar2=None,
                op0=mybir.AluOpType.mult,
            )
            tmp = small.tile([128, NEXP], F32)
            nc.vector.tensor_tensor(
                out=tmp, in0=r_sb, in1=negmask, op=mybir.AluOpType.add
            )
            m2 = small.tile([128, 1], F32)
            nc.vector.reduce_max(out=m2, in_=tmp, axis=mybir.AxisListType.X)
            # selected iff this expert's logit >= second max
            sel = small.tile([128, 1], F32)
            nc.vector.tensor_tensor(
                out=sel, in0=rl_sb, in1=m2, op=mybir.AluOpType.is_ge
            )
            rlm = small.tile([128, 1], F32)
            nc.vector.tensor_tensor(
                out=rlm, in0=rl_sb, in1=m1, op=mybir.AluOpType.subtract
            )
            m2m = small.tile([128, 1], F32)
            nc.vector.tensor_tensor(
                out=m2m, in0=m2, in1=m1, op=mybir.AluOpType.subtract
            )
            num = small.tile([128, 1], F32)
            nc.scalar.activation(
                out=num, in_=rlm, func=mybir.ActivationFunctionType.Exp,
            )
            den = small.tile([128, 1], F32)
            nc.scalar.activation(
                out=den, in_=m2m, func=mybir.ActivationFunctionType.Exp,
            )
            nc.vector.tensor_scalar(
                out=den, in0=den, scalar1=1.0, scalar2=None,
                op0=mybir.AluOpType.add,
            )
            rden = small.tile([128, 1], F32)
            nc.vector.reciprocal(out=rden, in_=den)
            coef = small.tile([128, 1], F32)
            nc.vector.tensor_tensor(
                out=coef, in0=num, in1=sel, op=mybir.AluOpType.mult
            )
            nc.vector.tensor_tensor(
                out=coef, in0=coef, in1=rden, op=mybir.AluOpType.mult
            )

            # ---- phase 1: MM1 + swiglu + transpose ----
            actT = singles.tile([128, KI, TOKENS], MM_DT)
            for b in range(IB):
                w1t = w1pool.tile([128, 2, KH, 512], MM_DT, tag="w1")
                eng = nc.sync if b % 2 == 0 else nc.scalar
                eng.dma_start(out=w1t, in_=w1s[b])
                wu = w1t[:, 0]
                wg = w1t[:, 1]

                pu = psum_u.tile([128, 512], F32)
                for k in range(KH):
                    nc.tensor.matmul(
                        pu, lhsT=_mm(hst_sb[:, k, :]), rhs=_mm(wu[:, k, :]),
                        start=(k == 0), stop=(k == KH - 1),
                    )
                pg = psum_g.tile([128, 512], F32)
                for k in range(KH):
                    nc.tensor.matmul(
                        pg, lhsT=_mm(hst_sb[:, k, :]), rhs=_mm(wg[:, k, :]),
                        start=(k == 0), stop=(k == KH - 1),
                    )
                act_b = actpool.tile([128, 512], F32)
                # silu(x) = x * sigmoid(x); CoreSim has no Silu LUT
                nc.scalar.activation(
                    out=act_b, in_=pu, func=mybir.ActivationFunctionType.Sigmoid
                )
                nc.vector.tensor_tensor(
                    out=act_b, in0=act_b, in1=pu, op=mybir.AluOpType.mult
                )
                nc.vector.tensor_tensor(
                    out=act_b, in0=act_b, in1=pg, op=mybir.AluOpType.mult
                )
                for j in range(4):
                    pt = psum_t.tile([128, 128], F32)
                    nc.tensor.transpose(pt, act_b[:, j * 128:(j + 1) * 128], ident)
                    nc.vector.tensor_copy(out=actT[:, b * 4 + j, :], in_=pt)

            # ---- phase 2: MM2 + coef scale ----
            for hb in range(HB):
                w2t = w2pool.tile([128, KI, HBW], MM_DT)
                eng = nc.sync if hb % 2 == 0 else nc.scalar
                eng.dma_start(out=w2t, in_=w2s[hb])
                py = psum_y.tile([128, HBW], F32)
                for ki in range(KI):
                    nc.tensor.matmul(
                        py, lhsT=_mm(actT[:, ki, :]), rhs=_mm(w2t[:, ki, :]),
                        start=(ki == 0), stop=(ki == KI - 1),
                    )
                yt = outpool.tile([128, HBW], F32)
                nc.vector.tensor_scalar(
                    out=yt, in0=py, scalar1=coef, scalar2=None,
                    op0=mybir.AluOpType.mult,
                )
                nc.sync.dma_start(
                    out=outp[:, hb * HBW:(hb + 1) * HBW], in_=yt
                )

    nc.finalize()
    return nc


_NC = None


def _get_nc():
    global _NC
    if _NC is None:
        _NC = build_bass()
    return _NC


def prep_inputs(hidden_states, routing, w1, w2):
    """Host-side shard + relayout. Returns in_maps for the 8 cores."""
    hs = np.ascontiguousarray(hidden_states, dtype=np.float32)
    rt = np.ascontiguousarray(routing, dtype=np.float32)
    w1 = np.asarray(w1, dtype=np.float32)
    w2 = np.asarray(w2, dtype=np.float32)

    # hst[p, k, t] = hs[t, k*128+p]
    hst = np.ascontiguousarray(hs.T.reshape(KH, 128, TOKENS).transpose(1, 0, 2))
    # w1s[e, b, p, u, k, o_l] = w1[e, u*4096 + b*512 + o_l, k*128 + p]
    w1p = np.ascontiguousarray(
        w1.reshape(NEXP, 2, IB, 512, KH, 128).transpose(0, 2, 5, 1, 4, 3)
    )
    # w2s[e, hb, p, ki, h_l] = w2[e, hb*HBW+h_l, ki*128+p]
    w2p = np.ascontiguousarray(
        w2.reshape(NEXP, HB, HBW, KI, 128).transpose(0, 1, 4, 3, 2)
    )

    in_maps = []
    for c in range(NCORES):
        in_maps.append({
            "hst": hst,
            "w1s": w1p[c],
            "w2s": w2p[c],
            "routing": rt,
            "rlogit": np.ascontiguousarray(rt[:, c:c + 1]),
        })
    return in_maps


def kernel(hidden_states, routing, w1, w2):
    nc = _get_nc()
    in_maps = prep_inputs(hidden_states, routing, w1, w2)
    res = run_bass_kernel_spmd(nc, in_maps, list(range(NCORES)))
    out = np.zeros((TOKENS, HIDDEN), dtype=np.float32)
    for c in range(NCORES):
        out += res.results[c]["outp"]
    return out
